# revision 7
# baseline (speedup 1.0000x reference)
"""Chebyshev (L-inf) pairwise distance matrix on 8 TRN2 NeuronCores.

reference: out[i, j] = max_d |embed1[i, d] - embed2[j, d]|
  embed1: [4096, 32] f32, embed2: [4096, 32] f32, out: [4096, 4096] f32

Sharding: 8 cores = 2 i-halves x 4 j-quarters. Each core computes the
[1024 j, 2048 i] transposed block of the output as 16 bf16 partial-max
slabs; the final 16-way max runs on the host (uint16-view max is
value-order-preserving for non-negative bf16).

Math: the level-1 pair reduction uses
    max(|x|, |y|) = 0.5|x+y| + 0.5|x-y|
with x = e1[i,2p]-e2[j,2p], y = e1[i,2p+1]-e2[j,2p+1]. The sum/diff
column transforms of e1 (pre-scaled by 0.5) and the matching e2 scalars
are host-precomputed, so on-chip work per d-pair is two absdiffs plus
one add:
  - absdiffs: ACT activation-Abs with per-partition bias (-e2 term), or
    DVE tensor_scalar subtract (4x bf16 perf mode) with a wide
    sign-clear AND afterwards (abs_max / fused arith+bitwise ops do not
    encode on core v3);
  - the add: DVE tensor_tensor (2x bf16) or the otherwise-idle GPSIMD
    (Pool) engine, which encodes tensor_tensor add.
This turns all 31 per-element max ops (DVE-only, 2x cap) into work
that three engines + host share.

Per-core layout: partition axis = j (8 blocks of 128), free axis = i
(w=2048). Group = 4 d-pairs (8 slots); per (j-block, group): produce
8 absdiff slots, add u+v, DMA the 4 pair-sums out.
"""

import sys

if "/opt/trn_rl_repo" not in sys.path:
    sys.path.insert(0, "/opt/trn_rl_repo")

from contextlib import ExitStack

import ml_dtypes
import numpy as np

import concourse.bacc as bacc
import concourse.bass as bass
import concourse.tile as tile
from concourse import mybir

BF16 = ml_dtypes.bfloat16

N = 4096          # rows of embed1 (= rows of embed2)
D = 32            # feature dim
NPAIR = D // 2    # 16 d-pairs -> 16 partial-max slabs
N_CORES = 8
N_IH = 2          # i split (embed1 rows)
N_JQ = 4          # j split (embed2 rows)
I_PER = N // N_IH       # 2048 per core
J_PER = N // N_JQ       # 1024 per core
JB = J_PER // 128       # 8 j-blocks per core
W = I_PER               # free-dim width per slot
NG = 8                  # groups per block (2 pairs each)
PPG = NPAIR // NG       # 2 pairs per group
SLOTS = 2 * PPG         # 4 slots per group (u0, u1, v0, v1)
GW = SLOTS * W          # free elems per group tile

# Production split within each group's 4 slots: ACT takes the first
# act_u u-slots and first act_v v-slots; DVE the rest (kept contiguous
# per u/v range so one wide sign-clear AND covers each). Two groups
# per block give ACT a 3rd slot -> 18 of 32 slots per block on ACT,
# which balances ACT (~2.0us/slot incl the 224-cycle SBUF bubble)
# against DVE (~1.3us/slot for 4x-mode sub + sign-clear, plus the 2x
# pair-adds). GPSIMD is NOT used: it shares SBUF ports with DVE, and
# a concurrent Pool tensor_tensor starves DVE ~20x (measured).
ACT_SPLIT = {g: ((2, 1) if g in (3, 7) else (2, 0)) for g in range(NG)}


_nc_cache = None


def _build_nc():
    nc = bacc.Bacc(
        trn_type="TRN2",
        target_bir_lowering=False,
        debug=False,
        num_devices=N_CORES,
    )

    dt_bf16 = mybir.dt.bfloat16
    dt_u16 = mybir.dt.uint16
    dt_f32 = mybir.dt.float32

    # e1 transformed (0.5*(a+b), 0.5*(a-b)) slabs, group-major, broadcast
    # across 128 partitions host-side: col = g*GW + slot*W + i.
    e1r = nc.declare_dram_parameter("e1r", [128, NG * GW], dt_bf16, isOutput=False)
    # e2 transform tables, [128, JB*D] f32: col = jb*D + slot,
    # slot in [0,8): group-local u then v scalars for that j-block.
    # Layout matches e1r group-major slot order: slot = g*8 + q.
    e2pos = nc.declare_dram_parameter("e2pos", [128, JB * D], dt_f32, isOutput=False)
    e2neg = nc.declare_dram_parameter("e2neg", [128, JB * D], dt_f32, isOutput=False)
    # 16 bf16 partial-max slabs: out[j, p*W + i] = 0.5|x+y| + 0.5|x-y|
    # for pair p; host maxes over p.
    out = nc.declare_dram_parameter("out", [J_PER, NPAIR * W], dt_bf16, isOutput=True)

    sub = mybir.AluOpType.subtract
    vadd = mybir.AluOpType.add
    band = mybir.AluOpType.bitwise_and

    with tile.TileContext(nc) as tc, ExitStack() as ctx:
        p_e1 = ctx.enter_context(tc.tile_pool(name="e1", bufs=1))
        p_e2 = ctx.enter_context(tc.tile_pool(name="e2", bufs=1))
        p_grp = ctx.enter_context(tc.tile_pool(name="grp", bufs=4))

        # --- one-time loads: e2 tables first (tiny), then e1 chunks ---
        t_e2p = p_e2.tile([128, JB * D], dt_f32, tag="e2p")
        t_e2n = p_e2.tile([128, JB * D], dt_f32, tag="e2n")
        nc.sync.dma_start(t_e2p[:], e2pos[:, :])
        nc.sync.dma_start(t_e2n[:], e2neg[:, :])

        t_e1g = []
        for g in range(NG):
            t = p_e1.tile([128, GW], dt_bf16, tag=f"e1g{g}")
            nc.sync.dma_start(t[:], e1r[:, g * GW:(g + 1) * GW])
            t_e1g.append(t)

        def produce(jb, g):
            """Emit the 4 absdiff slots for (jb, g); returns the group tile."""
            te1 = t_e1g[g]
            tg = p_grp.tile([128, GW], dt_bf16, tag="grp")
            act_u, act_v = ACT_SPLIT[g]

            def scalar_col(tbl, q):
                c = jb * D + g * SLOTS + q
                return tbl[:, c:c + 1]

            # DVE slots first (raw diffs; sign cleared below) so DVE
            # starts without waiting on ACT.
            dve_ranges = []
            if act_u < PPG:
                dve_ranges.append((act_u, PPG))
            if act_v < PPG:
                dve_ranges.append((PPG + act_v, SLOTS))
            for lo, hi in dve_ranges:
                for q in range(lo, hi):
                    nc.vector.tensor_scalar(
                        tg[:, q * W:(q + 1) * W],
                        te1[:, q * W:(q + 1) * W],
                        scalar_col(t_e2p, q),
                        None,
                        op0=sub,
                    )
            # wide sign-clear per contiguous DVE range
            for lo, hi in dve_ranges:
                r = tg[:, lo * W:hi * W].bitcast(dt_u16)
                nc.vector.tensor_scalar(r, r, 0x7FFF, None, op0=band)

            # ACT slots: fused absdiff, out = Abs(in * 1.0 + (-e2))
            for q in list(range(act_u)) + list(range(PPG, PPG + act_v)):
                nc.scalar.activation(
                    tg[:, q * W:(q + 1) * W],
                    te1[:, q * W:(q + 1) * W],
                    mybir.ActivationFunctionType.Abs,
                    bias=scalar_col(t_e2n, q),
                    scale=1.0,
                )
            return tg

        def finish(jb, g, tg):
            """Pair-add u += v, then store the pair-sums."""
            nc.vector.tensor_tensor(
                tg[:, :PPG * W], tg[:, :PPG * W], tg[:, PPG * W:], op=vadd
            )
            nc.sync.dma_start(
                out[jb * 128:(jb + 1) * 128,
                    (g * PPG) * W:(g + 1) * PPG * W],
                tg[:, :PPG * W],
            )

        # Group-major sweep: the whole first sweep needs only e1 chunk 0,
        # so production starts ~7us in while chunks 1..7 stream behind it
        # (block-major stalled ~20us catching up with the e1 DMA).
        # The add for each group is emitted one produce later, so DVE's
        # in-order queue doesn't block on ACT finishing the group it
        # just fed.
        pending = None
        for g in range(NG):
            for jb in range(JB):
                tg = produce(jb, g)
                if pending is not None:
                    finish(*pending)
                pending = (jb, g, tg)
        finish(*pending)

    nc.finalize()
    return nc


def _get_nc():
    global _nc_cache
    if _nc_cache is None:
        _nc_cache = _build_nc()
    return _nc_cache


def _transform_cols(x):
    """[n, 32] f32 -> [n, 32] f32: 16 scaled sums then 16 scaled diffs,
    in group-major slot order (g*8 + q; q<4 -> u=0.5(a+b), else v=0.5(a-b))."""
    a = x[:, 0::2]
    b = x[:, 1::2]
    u = 0.5 * (a + b)          # [n, 16] pair p = 2 cols (2p, 2p+1)
    v = 0.5 * (a - b)
    cols = np.empty_like(x)
    for g in range(NG):
        base = g * SLOTS
        pr = slice(g * PPG, (g + 1) * PPG)
        cols[:, base:base + PPG] = u[:, pr]
        cols[:, base + PPG:base + SLOTS] = v[:, pr]
    return cols


def make_in_maps(embed1: np.ndarray, embed2: np.ndarray):
    """Host-side sharding/prep. Returns in_maps for cores 0..7.

    Core c: ih = c % N_IH, jq = c // N_IH.
    """
    embed1 = np.asarray(embed1, dtype=np.float32)
    embed2 = np.asarray(embed2, dtype=np.float32)
    in_maps = []
    for c in range(N_CORES):
        ih, jq = c % N_IH, c // N_IH
        e1_slab = embed1[ih * I_PER:(ih + 1) * I_PER, :]      # [2048, 32]
        e1_t = _transform_cols(e1_slab)                       # [2048, 32]
        # slot-major flatten (col s*W + i), bf16, broadcast to 128 parts
        flat = np.ascontiguousarray(e1_t.T).reshape(-1).astype(BF16)
        rep = np.ascontiguousarray(
            np.broadcast_to(flat[None, :], (128, D * W)))
        e2_slab = embed2[jq * J_PER:(jq + 1) * J_PER, :]      # [1024, 32]
        e2_t = _transform_cols(e2_slab)                       # [1024, 32]
        # [128, JB*D] f32: partition p, col jb*D + slot -> e2_t[jb*128+p, slot]
        e2_tbl = np.ascontiguousarray(
            e2_t.reshape(JB, 128, D).transpose(1, 0, 2).reshape(128, JB * D))
        in_maps.append({
            "e1r": rep,
            "e2pos": e2_tbl,
            "e2neg": np.ascontiguousarray(-e2_tbl),
        })
    return in_maps


def assemble(results) -> np.ndarray:
    """results: per-core dicts with 'out' [J_PER, NPAIR*W] bf16 partials."""
    full = np.empty((N, N), dtype=np.float32)
    for c in range(N_CORES):
        ih, jq = c % N_IH, c // N_IH
        part = np.asarray(results[c]["out"])          # [1024, 16*2048] bf16
        # non-negative bf16: value order == uint16 order
        pu = part.view(np.uint16).reshape(J_PER, NPAIR, W)
        red = np.max(pu, axis=1)                       # [1024, 2048] u16
        blk = red.view(BF16).astype(np.float32)        # [j, i]
        full[ih * I_PER:(ih + 1) * I_PER,
             jq * J_PER:(jq + 1) * J_PER] = blk.T
    return full


def kernel(embed1: np.ndarray, embed2: np.ndarray) -> np.ndarray:
    from concourse.bass_utils import run_bass_kernel_spmd

    nc = _get_nc()
    in_maps = make_in_maps(np.asarray(embed1), np.asarray(embed2))
    res = run_bass_kernel_spmd(nc, in_maps, core_ids=list(range(N_CORES)))
    return assemble(res.results)


if __name__ == "__main__":
    e1 = np.random.randn(N, D).astype(np.float32)
    e2 = np.random.randn(N, D).astype(np.float32)
    out = kernel(embed1=e1, embed2=e2)
    ref = np.max(np.abs(e1[:, None, :] - e2[None, :, :]), axis=2)
    err = np.abs(out - ref).max() / np.abs(ref).max()
    print("rel err:", err)


# revision 9
# speedup vs baseline: 1.1231x; 1.1231x over previous
"""Chebyshev (L-inf) pairwise distance matrix on 8 TRN2 NeuronCores.

reference: out[i, j] = max_d |embed1[i, d] - embed2[j, d]|
  embed1: [4096, 32] f32, embed2: [4096, 32] f32, out: [4096, 4096] f32

Sharding: 8 cores = 2 i-halves x 4 j-quarters. Each core computes the
[1024 j, 2048 i] transposed block of the output as 16 bf16 partial-max
slabs; the final 16-way max runs on the host (uint16-view max is
value-order-preserving for non-negative bf16).

Math: the level-1 pair reduction uses
    max(|x|, |y|) = 0.5|x+y| + 0.5|x-y|
with x = e1[i,2p]-e2[j,2p], y = e1[i,2p+1]-e2[j,2p+1]. The sum/diff
column transforms of e1 (pre-scaled by 0.5) and the matching e2 scalars
are host-precomputed, so on-chip work per d-pair is two absdiffs plus
one add:
  - absdiffs: ACT activation-Abs with per-partition bias (-e2 term), or
    DVE tensor_scalar subtract (4x bf16 perf mode) with a wide
    sign-clear AND afterwards (abs_max / fused arith+bitwise ops do not
    encode on core v3);
  - the add: DVE tensor_tensor (2x bf16) or the otherwise-idle GPSIMD
    (Pool) engine, which encodes tensor_tensor add.
This turns all 31 per-element max ops (DVE-only, 2x cap) into work
that three engines + host share.

Per-core layout: partition axis = j (8 blocks of 128), free axis = i
(w=2048). Group = 4 d-pairs (8 slots); per (j-block, group): produce
8 absdiff slots, add u+v, DMA the 4 pair-sums out.
"""

import sys

if "/opt/trn_rl_repo" not in sys.path:
    sys.path.insert(0, "/opt/trn_rl_repo")

from contextlib import ExitStack

import ml_dtypes
import numpy as np

import concourse.bacc as bacc
import concourse.bass as bass
import concourse.tile as tile
from concourse import mybir

BF16 = ml_dtypes.bfloat16

N = 4096          # rows of embed1 (= rows of embed2)
D = 32            # feature dim
NPAIR = D // 2    # 16 d-pairs -> 16 partial-max slabs
N_CORES = 8
N_IH = 2          # i split (embed1 rows)
N_JQ = 4          # j split (embed2 rows)
I_PER = N // N_IH       # 2048 per core
J_PER = N // N_JQ       # 1024 per core
JB = J_PER // 128       # 8 j-blocks per core
W = I_PER               # free-dim width per slot
NG = 8                  # groups per block (2 pairs each)
PPG = NPAIR // NG       # 2 pairs per group
SLOTS = 2 * PPG         # 4 slots per group (u0, u1, v0, v1)
GW = SLOTS * W          # free elems per group tile

# Production split within each group's 4 slots: ACT takes the first
# act_u u-slots and first act_v v-slots; DVE the rest (kept contiguous
# per u/v range so one wide sign-clear AND covers each). Two groups
# per block give ACT a 3rd slot -> 18 of 32 slots per block on ACT,
# which balances ACT (~2.0us/slot incl the 224-cycle SBUF bubble)
# against DVE (~1.3us/slot for 4x-mode sub + sign-clear, plus the 2x
# pair-adds). GPSIMD is NOT used: it shares SBUF ports with DVE, and
# a concurrent Pool tensor_tensor starves DVE ~20x (measured).
ACT_SPLIT = {g: ((2, 1) if g in (3, 7) else (2, 0)) for g in range(NG)}


_nc_cache = None


def _build_nc():
    nc = bacc.Bacc(
        trn_type="TRN2",
        target_bir_lowering=False,
        debug=False,
        num_devices=N_CORES,
    )

    dt_bf16 = mybir.dt.bfloat16
    dt_u16 = mybir.dt.uint16
    dt_f32 = mybir.dt.float32

    # e1 transformed (0.5*(a+b), 0.5*(a-b)) slabs, group-major, broadcast
    # across 128 partitions host-side: col = g*GW + slot*W + i.
    e1r = nc.declare_dram_parameter("e1r", [128, NG * GW], dt_bf16, isOutput=False)
    # e2 transform tables, [128, JB*D] f32: col = jb*D + slot,
    # slot in [0,8): group-local u then v scalars for that j-block.
    # Layout matches e1r group-major slot order: slot = g*8 + q.
    e2pos = nc.declare_dram_parameter("e2pos", [128, JB * D], dt_f32, isOutput=False)
    e2neg = nc.declare_dram_parameter("e2neg", [128, JB * D], dt_f32, isOutput=False)
    # 16 bf16 partial-max slabs: out[j, p*W + i] = 0.5|x+y| + 0.5|x-y|
    # for pair p; host maxes over p.
    out = nc.declare_dram_parameter("out", [J_PER, NPAIR * W], dt_bf16, isOutput=True)

    sub = mybir.AluOpType.subtract
    vadd = mybir.AluOpType.add
    band = mybir.AluOpType.bitwise_and

    with tile.TileContext(nc) as tc, ExitStack() as ctx:
        # In the group-major sweep each e1 chunk is dead after its own
        # sweep, so e1 rotates through 3 buffers (48 KiB) instead of
        # sitting fully resident (128 KiB); the freed SBUF pays for 6
        # group buffers + separate DMA-out staging tiles, which takes
        # the out-DMA completion latency off the production WAR chain.
        p_e1 = ctx.enter_context(tc.tile_pool(name="e1", bufs=3))
        p_e2 = ctx.enter_context(tc.tile_pool(name="e2", bufs=1))
        p_grp = ctx.enter_context(tc.tile_pool(name="grp", bufs=6))
        p_sum = ctx.enter_context(tc.tile_pool(name="sum", bufs=6))

        # --- loads: e2 tables first (tiny), then the first e1 chunks;
        # chunk g+2 is issued at the start of sweep g (3-buffer rotation).
        t_e2p = p_e2.tile([128, JB * D], dt_f32, tag="e2p")
        t_e2n = p_e2.tile([128, JB * D], dt_f32, tag="e2n")
        nc.sync.dma_start(t_e2p[:], e2pos[:, :])
        nc.sync.dma_start(t_e2n[:], e2neg[:, :])

        def load_chunk(g):
            t = p_e1.tile([128, GW], dt_bf16, tag="e1g")
            if g < 2:
                # split so the DVE-owned v-slots land first: production
                # (which starts with the v subs) begins ~halfway through
                # the chunk transfer
                nc.sync.dma_start(t[:, PPG * W:], e1r[:, g * GW + PPG * W:(g + 1) * GW])
                nc.sync.dma_start(t[:, :PPG * W], e1r[:, g * GW:g * GW + PPG * W])
            else:
                nc.sync.dma_start(t[:], e1r[:, g * GW:(g + 1) * GW])
            return t

        t_e1g = {g: load_chunk(g) for g in range(2)}

        def produce(jb, g):
            """Emit the 4 absdiff slots for (jb, g); returns the group tile."""
            te1 = t_e1g[g]
            tg = p_grp.tile([128, GW], dt_bf16, tag="grp")
            act_u, act_v = ACT_SPLIT[g]

            def scalar_col(tbl, q):
                c = jb * D + g * SLOTS + q
                return tbl[:, c:c + 1]

            # DVE slots first (raw diffs; sign cleared below) so DVE
            # starts without waiting on ACT.
            dve_ranges = []
            if act_u < PPG:
                dve_ranges.append((act_u, PPG))
            if act_v < PPG:
                dve_ranges.append((PPG + act_v, SLOTS))
            for lo, hi in dve_ranges:
                for q in range(lo, hi):
                    nc.vector.tensor_scalar(
                        tg[:, q * W:(q + 1) * W],
                        te1[:, q * W:(q + 1) * W],
                        scalar_col(t_e2p, q),
                        None,
                        op0=sub,
                    )
            # wide sign-clear per contiguous DVE range
            for lo, hi in dve_ranges:
                r = tg[:, lo * W:hi * W].bitcast(dt_u16)
                nc.vector.tensor_scalar(r, r, 0x7FFF, None, op0=band)

            # ACT slots: fused absdiff, out = Abs(in * 1.0 + (-e2))
            for q in list(range(act_u)) + list(range(PPG, PPG + act_v)):
                nc.scalar.activation(
                    tg[:, q * W:(q + 1) * W],
                    te1[:, q * W:(q + 1) * W],
                    mybir.ActivationFunctionType.Abs,
                    bias=scalar_col(t_e2n, q),
                    scale=1.0,
                )
            return tg

        def finish(jb, g, tg):
            """Pair-add into a staging tile, then store the pair-sums.
            Writing the add to a separate tile frees the group tile at
            the add itself (DVE-internal), not at DMA completion."""
            ts = p_sum.tile([128, PPG * W], dt_bf16, tag="sum")
            nc.vector.tensor_tensor(
                ts[:], tg[:, :PPG * W], tg[:, PPG * W:], op=vadd
            )
            nc.sync.dma_start(
                out[jb * 128:(jb + 1) * 128,
                    (g * PPG) * W:(g + 1) * PPG * W],
                ts[:],
            )

        # Group-major sweep: the whole first sweep needs only e1 chunk 0,
        # so production starts while chunks 1..7 stream behind it.
        # The add for each group is emitted one produce later, so DVE's
        # in-order queue doesn't block on ACT finishing the group it
        # just fed.
        pending = None
        for g in range(NG):
            if g + 2 < NG:
                t_e1g[g + 2] = load_chunk(g + 2)
            for jb in range(JB):
                tg = produce(jb, g)
                if pending is not None:
                    finish(*pending)
                pending = (jb, g, tg)
        finish(*pending)

    nc.finalize()
    return nc


def _get_nc():
    global _nc_cache
    if _nc_cache is None:
        _nc_cache = _build_nc()
    return _nc_cache


def _transform_cols(x):
    """[n, 32] f32 -> [n, 32] f32: 16 scaled sums then 16 scaled diffs,
    in group-major slot order (g*8 + q; q<4 -> u=0.5(a+b), else v=0.5(a-b))."""
    a = x[:, 0::2]
    b = x[:, 1::2]
    u = 0.5 * (a + b)          # [n, 16] pair p = 2 cols (2p, 2p+1)
    v = 0.5 * (a - b)
    cols = np.empty_like(x)
    for g in range(NG):
        base = g * SLOTS
        pr = slice(g * PPG, (g + 1) * PPG)
        cols[:, base:base + PPG] = u[:, pr]
        cols[:, base + PPG:base + SLOTS] = v[:, pr]
    return cols


def make_in_maps(embed1: np.ndarray, embed2: np.ndarray):
    """Host-side sharding/prep. Returns in_maps for cores 0..7.

    Core c: ih = c % N_IH, jq = c // N_IH.
    """
    embed1 = np.asarray(embed1, dtype=np.float32)
    embed2 = np.asarray(embed2, dtype=np.float32)
    in_maps = []
    for c in range(N_CORES):
        ih, jq = c % N_IH, c // N_IH
        e1_slab = embed1[ih * I_PER:(ih + 1) * I_PER, :]      # [2048, 32]
        e1_t = _transform_cols(e1_slab)                       # [2048, 32]
        # slot-major flatten (col s*W + i), bf16, broadcast to 128 parts
        flat = np.ascontiguousarray(e1_t.T).reshape(-1).astype(BF16)
        rep = np.ascontiguousarray(
            np.broadcast_to(flat[None, :], (128, D * W)))
        e2_slab = embed2[jq * J_PER:(jq + 1) * J_PER, :]      # [1024, 32]
        e2_t = _transform_cols(e2_slab)                       # [1024, 32]
        # [128, JB*D] f32: partition p, col jb*D + slot -> e2_t[jb*128+p, slot]
        e2_tbl = np.ascontiguousarray(
            e2_t.reshape(JB, 128, D).transpose(1, 0, 2).reshape(128, JB * D))
        in_maps.append({
            "e1r": rep,
            "e2pos": e2_tbl,
            "e2neg": np.ascontiguousarray(-e2_tbl),
        })
    return in_maps


def assemble(results) -> np.ndarray:
    """results: per-core dicts with 'out' [J_PER, NPAIR*W] bf16 partials."""
    full = np.empty((N, N), dtype=np.float32)
    for c in range(N_CORES):
        ih, jq = c % N_IH, c // N_IH
        part = np.asarray(results[c]["out"])          # [1024, 16*2048] bf16
        # non-negative bf16: value order == uint16 order
        pu = part.view(np.uint16).reshape(J_PER, NPAIR, W)
        red = np.max(pu, axis=1)                       # [1024, 2048] u16
        blk = red.view(BF16).astype(np.float32)        # [j, i]
        full[ih * I_PER:(ih + 1) * I_PER,
             jq * J_PER:(jq + 1) * J_PER] = blk.T
    return full


def kernel(embed1: np.ndarray, embed2: np.ndarray) -> np.ndarray:
    from concourse.bass_utils import run_bass_kernel_spmd

    nc = _get_nc()
    in_maps = make_in_maps(np.asarray(embed1), np.asarray(embed2))
    res = run_bass_kernel_spmd(nc, in_maps, core_ids=list(range(N_CORES)))
    return assemble(res.results)


if __name__ == "__main__":
    e1 = np.random.randn(N, D).astype(np.float32)
    e2 = np.random.randn(N, D).astype(np.float32)
    out = kernel(embed1=e1, embed2=e2)
    ref = np.max(np.abs(e1[:, None, :] - e2[None, :, :]), axis=2)
    err = np.abs(out - ref).max() / np.abs(ref).max()
    print("rel err:", err)
